# revision 36
# baseline (speedup 1.0000x reference)
"""Trainium2 Bass kernel for nn_MemoryWriter (scatter_memory).

Math (see reference):
    w        = where(gate > 0.01, gate * 0.1, 0)            [B]
    contrib  (q_a, v_a, w_a) scattered to slots top_indices[a, :]
    upd_k[s] = sum_j w_j q_j / (counts>0 ? counts : 1), counts = sum_j w_j
    out_k    = mem_k + 0.9 * mom_k + (1 - 0.9) * upd_k      (mom is zeros)

Sharding: slot dimension across 8 cores (8192 slots each).  The host performs
the contribution routing that the all-to-all performs in a real distributed
setting (per the sharding hint).  Because each slot lives on exactly one core,
the per-slot weight sums (counts) are host-computable during routing, so the
routed scatter weights are PRE-DIVIDED: oh[r, s] = (1-momentum) * w_r / denom_s.
The device work per 128-slot tile is then just:

    psum = oh_inc.T @ qv_inc  (+ further fragments)    # PE fp8 matmul scatter
    out_tile = psum + mem_tile  -> bf16                # drain+add

The drain is split per 4-tile PSUM group: tiles 0-1 drain on the DVE
(tensor_tensor add straight from PSUM), tiles 2-3 get the mem tile added by
an identity matmul on the PE and drain via an ACT copy — balancing PE/DVE/ACT.

All device inputs are packed host-side into ONE DRAM buffer per core laid out
as the exact SBUF image [128 partitions, bytes] = per chunk [mem|qv|oh], so
the whole input side is 5 large fully-contiguous DMAs.  The memory table
flows through the device in bf16 (rel err ~2e-3, tolerance 2e-2);
contributions in fp8e4m3; the host casts the bf16 output table back to f32.
"""

import numpy as np

# ---- problem constants (hardcoded per contest contract) --------------------
N_SLOTS = 65536
DIM = 128
B = 4096
K = 8
NCORES = 8
SPC = N_SLOTS // NCORES      # slots per core = 8192
NT = SPC // 128              # slot tiles per core = 64
P = 128
GATE_THRESH = 0.01
UPDATE_RATE = 0.1
MOMENTUM = 0.9
UPD = float(np.float32(1.0) - np.float32(MOMENTUM))

GT = 2                       # slot tiles per PSUM group (one bank per tile)
LD_BOUNDS = [0, 4, 12, 28, 44, 60, 64]   # load-chunk tile boundaries
ST_BOUNDS = [0, 16, 32, 48, 56, 60, 64]  # store-chunk tile boundaries (%GT==0)
MEM_SCALE = 6.0 / 127.0      # int8 memory-table encoding: mem ~= s * q

_BUILD_CACHE = {}


def _act_route(t):
    """Most PSUM groups drain on the DVE (int8 mem, fused add); every third
    group adds the bf16 mem tile on the PE (identity matmul), drains on ACT."""
    return (t // GT) % 3 == 2


def _layout(Fs):
    """Byte layout of the combined per-core input image.

    Per load chunk: [scale/ident (chunk 0) | mem (256B int8 or 512B bf16 per
    tile, by drain route) | qv 256B/inc | oh 128B/inc] per partition.
    Returns (total_bytes, per-chunk bases, mem_off per tile, inc_off).
    """
    inc_off = [0]
    for f in Fs:
        inc_off.append(inc_off[-1] + f)
    chunks = []
    mem_off = [0] * NT
    base = 0
    for ci in range(len(LD_BOUNDS) - 1):
        t0, t1 = LD_BOUNDS[ci], LD_BOUNDS[ci + 1]
        i0, i1 = inc_off[t0], inc_off[t1]
        mem_b = base + ((4 + 256) if ci == 0 else 0)  # chunk 0: scale + ident
        pos = mem_b
        for t in range(t0, t1):
            mem_off[t] = pos
            pos += 512 if _act_route(t) else 256
        qv_b = pos
        oh_b = qv_b + (i1 - i0) * 256
        end = oh_b + (i1 - i0) * 128
        chunks.append((mem_b, qv_b, oh_b, end))
        base = end
    return base, chunks, mem_off, inc_off


def build_nc(Fs):
    """Build the per-core Bass program.

    Fs: per slot-tile fragment counts (ceil(max-count-over-cores / 128)),
    shared across cores so one program serves all 8.
    """
    import concourse.bacc as bacc
    import concourse.tile as tile
    from concourse import mybir
    from contextlib import ExitStack

    f32 = mybir.dt.float32
    bf16 = mybir.dt.bfloat16
    fp8 = mybir.dt.float8e4
    u8 = mybir.dt.uint8
    i8 = mybir.dt.int8
    Alu = mybir.AluOpType

    TOT, chunks, mem_off, inc_off = _layout(Fs)
    assert all(b % GT == 0 for b in LD_BOUNDS + ST_BOUNDS)

    nc = bacc.Bacc("TRN2", target_bir_lowering=False, debug=False)

    img_in = nc.dram_tensor("img", [P, TOT], u8, kind="ExternalInput")
    out_kv = nc.dram_tensor("out_kv", [P, NT * 256], bf16, kind="ExternalOutput")

    # view helpers: tile t lives in chunk ch(t); incidence inc in chunk of its tile
    def chunk_of(t):
        for ci in range(len(LD_BOUNDS) - 1):
            if LD_BOUNDS[ci] <= t < LD_BOUNDS[ci + 1]:
                return ci
        raise AssertionError

    with tile.TileContext(nc) as tc, ExitStack() as ctx:
        pool = ctx.enter_context(tc.tile_pool(name="main", bufs=1))
        pspool = ctx.enter_context(tc.tile_pool(name="ps", bufs=4, space="PSUM"))

        img_t = pool.tile([P, TOT], u8)
        out_t = pool.tile([P, NT * 256], bf16)

        prev = 0
        for (mem_b, qv_b, oh_b, end) in chunks:
            nc.sync.dma_start(img_t[:, prev:end], img_in[:, prev:end])
            prev = end
        scale_ap = img_t[:, 0:4].bitcast(f32)      # [p, 1] = MEM_SCALE
        ident_t = img_t[:, 4:260].bitcast(bf16)    # [p, 128] identity

        def mem_view(t, n=1):
            # n tiles starting at t; all same route (route is per-group)
            off = mem_off[t]
            if _act_route(t):
                return img_t[:, off:off + n * 512].bitcast(bf16)
            return img_t[:, off:off + n * 256].bitcast(i8)

        def qv_view(t, fi):
            ci = chunk_of(t)
            qv_b = chunks[ci][1]
            off = qv_b + (inc_off[t] + fi - inc_off[LD_BOUNDS[ci]]) * 256
            return img_t[:, off:off + 256].bitcast(fp8)

        def oh_view(t, fi):
            ci = chunk_of(t)
            oh_b = chunks[ci][2]
            off = oh_b + (inc_off[t] + fi - inc_off[LD_BOUNDS[ci]]) * 128
            return img_t[:, off:off + 128].bitcast(fp8)

        st_done = 0
        for g in range(NT // GT):
            # one PSUM bank per tile ("start" zeroing operates on the whole
            # bank, so accumulation tiles must not share banks)
            act_route = _act_route(g * GT)
            ps = pspool.tile([P, GT * 512], f32, tag="ps")
            ps3 = ps[:].rearrange("p (i c) -> p i c", c=512)
            for i in range(GT):
                t = g * GT + i
                slc = ps[:, i * 512:i * 512 + 256]
                for fi in range(Fs[t]):
                    nc.tensor.matmul(
                        slc, lhsT=oh_view(t, fi), rhs=qv_view(t, fi),
                        start=(fi == 0),
                        stop=(not act_route and fi == Fs[t] - 1),
                    )
            c0 = g * GT * 256
            dst = out_t[:, c0:c0 + GT * 256].rearrange("p (i c) -> p i c", c=256)
            if act_route:
                # bf16 mem rides the PE, two tiles per strided matmul
                for h in range(GT // 2):
                    nc.tensor.matmul(
                        ps3[:, 2 * h:2 * h + 2, 0:256], lhsT=ident_t,
                        rhs=mem_view(g * GT + 2 * h, 2),
                        start=False, stop=True,
                    )
                nc.scalar.copy(dst, ps3[:, :, 0:256])
            else:
                # drain: out = s * mem_i8 + psum, fused on the DVE
                memv = mem_view(g * GT, GT).rearrange("p (i c) -> p i c", c=256)
                nc.vector.scalar_tensor_tensor(
                    dst, memv, scale_ap, ps3[:, :, 0:256],
                    op0=Alu.mult, op1=Alu.add)

            tend = (g + 1) * GT
            if st_done < len(ST_BOUNDS) - 1 and tend == ST_BOUNDS[st_done + 1]:
                t0, t1 = ST_BOUNDS[st_done], ST_BOUNDS[st_done + 1]
                nc.scalar.dma_start(
                    out_kv[:, t0 * 256:t1 * 256], out_t[:, t0 * 256:t1 * 256])
                st_done += 1

    nc.compile()
    return nc


def prepare_inputs(inputs):
    """Host-side routing (the all-to-all stand-in): bucket contributions by
    (core, slot-tile), pre-divide weights by the local per-slot weight sums,
    and materialize each core's combined SBUF-image buffer."""
    import ml_dtypes
    bf16 = ml_dtypes.bfloat16
    fp8 = ml_dtypes.float8_e4m3

    mk = np.asarray(inputs["memory_keys"], dtype=np.float32)
    mv = np.asarray(inputs["memory_values"], dtype=np.float32)
    q = np.asarray(inputs["write_query"], dtype=np.float32)
    v = np.asarray(inputs["write_value"], dtype=np.float32)
    gate = np.asarray(inputs["gate_weights"], dtype=np.float32)
    ti = np.asarray(inputs["top_indices"]).astype(np.int64).reshape(-1)

    w = np.where(gate > GATE_THRESH, gate * np.float32(UPDATE_RATE),
                 np.float32(0.0)).astype(np.float32)
    wk = np.repeat(w, K)                                     # [B*K]
    cnt = np.bincount(ti, weights=wk.astype(np.float64),
                      minlength=N_SLOTS).astype(np.float32)
    denom = np.where(cnt > 0, cnt, np.float32(1.0)).astype(np.float32)
    ohv = (np.float32(UPD) * wk / denom[ti]).astype(np.float32)

    a = np.arange(B * K, dtype=np.int64) // K
    gtile = ti >> 7                                          # global tile id
    order = np.argsort(gtile, kind="stable")
    g_s = gtile[order]
    a_s = a[order]
    s_s = (ti & 127)[order]
    ohv_s = ohv[order]
    cnt_pt = np.bincount(gtile, minlength=NCORES * NT)
    starts = np.zeros(NCORES * NT + 1, dtype=np.int64)
    starts[1:] = np.cumsum(cnt_pt)
    rowpos = np.arange(B * K, dtype=np.int64) - starts[g_s]

    cnt2 = cnt_pt.reshape(NCORES, NT)
    cnt_max = cnt2.max(axis=0)
    Fs = tuple(int(max(1, -(-c // 128))) for c in cnt_max)
    inc_off = np.zeros(NT + 1, dtype=np.int64)
    inc_off[1:] = np.cumsum(Fs)
    NINC = int(inc_off[-1])

    core_s = g_s >> 6
    t_s = g_s & 63
    inc_s = inc_off[t_s] + (rowpos >> 7)
    p_s = rowpos & 127

    qv_full = np.concatenate([q, v], axis=1)                 # [B, 256]
    qv_img = np.zeros((NCORES, P, NINC * 256), dtype=np.float32)
    oh_img = np.zeros((NCORES, P, NINC * 128), dtype=np.float32)
    oh_img[core_s, p_s, inc_s * 128 + s_s] = ohv_s
    cols = (inc_s * 256)[:, None] + np.arange(256)[None, :]
    qv_img[core_s[:, None], p_s[:, None], cols] = qv_full[a_s]
    qv_u8 = qv_img.astype(fp8).view(np.uint8)                # [C, P, NINC*256]
    oh_u8 = oh_img.astype(fp8).view(np.uint8)                # [C, P, NINC*128]

    mkv = np.concatenate([mk, mv], axis=1)                   # [65536, 256]
    # per-tile [C, P, 256] views in both encodings
    mem_t = np.ascontiguousarray(
        mkv.reshape(NCORES, NT, P, 256).transpose(0, 2, 1, 3))  # [C,P,NT,256]
    # int8 encoding (DVE route); |mem| > 6 clips, patched on host in kernel()
    mem_i8 = np.clip(np.round(mem_t / np.float32(MEM_SCALE)), -127, 127
                     ).astype(np.int8).view(np.uint8)
    mem_b16 = mem_t.astype(bf16).view(np.uint8)              # [C,P,NT,512]

    scale_u8 = np.broadcast_to(
        np.full((1, P, 1), MEM_SCALE, dtype=np.float32).view(np.uint8),
        (NCORES, P, 4))
    ident_u8 = np.broadcast_to(
        np.eye(P, dtype=bf16).view(np.uint8)[None], (NCORES, P, 256))
    parts = [scale_u8, ident_u8]
    for ci in range(len(LD_BOUNDS) - 1):
        t0, t1 = LD_BOUNDS[ci], LD_BOUNDS[ci + 1]
        i0, i1 = int(inc_off[t0]), int(inc_off[t1])
        for t in range(t0, t1):
            parts.append(mem_b16[:, :, t] if _act_route(t) else mem_i8[:, :, t])
        parts.append(qv_u8[:, :, i0 * 256:i1 * 256])
        parts.append(oh_u8[:, :, i0 * 128:i1 * 128])
    img = np.concatenate(parts, axis=2)                      # [C, P, TOT]

    in_maps = []
    for c in range(NCORES):
        in_maps.append({
            "img": np.ascontiguousarray(img[c]),
        })
    return in_maps, Fs


def kernel(**inputs):
    from concourse.bass_utils import run_bass_kernel_spmd

    in_maps, Fs = prepare_inputs(inputs)
    if Fs not in _BUILD_CACHE:
        _BUILD_CACHE[Fs] = build_nc(Fs)
    nc = _BUILD_CACHE[Fs]

    res = run_bass_kernel_spmd(nc, in_maps, core_ids=list(range(NCORES)))
    out_img = np.stack([res.results[c]["out_kv"] for c in range(NCORES)])
    # un-permute the SBUF image layout: [c, p, t*256+d] -> [c*8192+t*128+p, d]
    out_kv = np.ascontiguousarray(
        out_img.reshape(NCORES, P, NT, 256).transpose(0, 2, 1, 3)
    ).reshape(N_SLOTS, 256).astype(np.float32)
    # sparse clip patch: int8 encoding clips |mem| > 6; restore those entries
    # (out = mem + upd is linear in mem, so adding the clip residual is exact)
    mk = np.asarray(inputs["memory_keys"], dtype=np.float32)
    mv = np.asarray(inputs["memory_values"], dtype=np.float32)
    mkv = np.concatenate([mk, mv], axis=1)
    rows = np.unique(np.nonzero(np.abs(mkv) > 126.5 * MEM_SCALE)[0])
    rows = rows[~np.vectorize(_act_route)(rows // 128 % NT)] if rows.size else rows
    if rows.size:
        dec = np.float32(MEM_SCALE) * np.clip(
            np.round(mkv[rows] / np.float32(MEM_SCALE)), -127, 127)
        out_kv[rows] += mkv[rows] - dec

    out_k = np.ascontiguousarray(out_kv[:, 0:DIM])
    out_v = np.ascontiguousarray(out_kv[:, DIM:2 * DIM])

    km = np.asarray(inputs["key_momentum"], dtype=np.float32)
    vm = np.asarray(inputs["value_momentum"], dtype=np.float32)
    # mom is zeros in this problem; fall back to a host-side add if it isn't
    if np.any(km):
        out_k = out_k + np.float32(MOMENTUM) * km
    if np.any(vm):
        out_v = out_v + np.float32(MOMENTUM) * vm
    return out_k, out_v


# revision 37
# speedup vs baseline: 1.0427x; 1.0427x over previous
"""Trainium2 Bass kernel for nn_MemoryWriter (scatter_memory).

Math (see reference):
    w        = where(gate > 0.01, gate * 0.1, 0)            [B]
    contrib  (q_a, v_a, w_a) scattered to slots top_indices[a, :]
    upd_k[s] = sum_j w_j q_j / (counts>0 ? counts : 1), counts = sum_j w_j
    out_k    = mem_k + 0.9 * mom_k + (1 - 0.9) * upd_k      (mom is zeros)

Sharding: slot dimension across 8 cores (8192 slots each).  The host performs
the contribution routing that the all-to-all performs in a real distributed
setting (per the sharding hint).  Because each slot lives on exactly one core,
the per-slot weight sums (counts) are host-computable during routing, so the
routed scatter weights are PRE-DIVIDED: oh[r, s] = (1-momentum) * w_r / denom_s.
The device work per 128-slot tile is then just:

    psum = oh_inc.T @ qv_inc  (+ further fragments)    # PE fp8 matmul scatter
    out_tile = psum + mem_tile  -> bf16                # drain+add

The drain is split per 4-tile PSUM group: tiles 0-1 drain on the DVE
(tensor_tensor add straight from PSUM), tiles 2-3 get the mem tile added by
an identity matmul on the PE and drain via an ACT copy — balancing PE/DVE/ACT.

All device inputs are packed host-side into ONE DRAM buffer per core laid out
as the exact SBUF image [128 partitions, bytes] = per chunk [mem|qv|oh], so
the whole input side is 5 large fully-contiguous DMAs.  The memory table
flows through the device in bf16 (rel err ~2e-3, tolerance 2e-2);
contributions in fp8e4m3; the host casts the bf16 output table back to f32.
"""

import numpy as np

# ---- problem constants (hardcoded per contest contract) --------------------
N_SLOTS = 65536
DIM = 128
B = 4096
K = 8
NCORES = 8
SPC = N_SLOTS // NCORES      # slots per core = 8192
NT = SPC // 128              # slot tiles per core = 64
P = 128
GATE_THRESH = 0.01
UPDATE_RATE = 0.1
MOMENTUM = 0.9
UPD = float(np.float32(1.0) - np.float32(MOMENTUM))

GT = 2                       # slot tiles per PSUM group (one bank per tile)
LD_BOUNDS = [0, 4, 12, 28, 44, 60, 64]   # load-chunk tile boundaries
ST_BOUNDS = [0, 16, 32, 48, 64]          # store-chunk tile boundaries (%GT==0)
MEM_SCALE = 6.0 / 127.0      # int8 memory-table encoding: mem ~= s * q

_BUILD_CACHE = {}


def _act_route(t):
    """Most PSUM groups drain on the DVE (int8 mem, fused add); every third
    group adds the bf16 mem tile on the PE (identity matmul), drains on ACT."""
    return (t // GT) % 3 == 2


def _layout(Fs):
    """Byte layout of the combined per-core input image.

    Per load chunk: [scale/ident (chunk 0) | mem (256B int8 or 512B bf16 per
    tile, by drain route) | qv 256B/inc | oh 128B/inc] per partition.
    Returns (total_bytes, per-chunk bases, mem_off per tile, inc_off).
    """
    inc_off = [0]
    for f in Fs:
        inc_off.append(inc_off[-1] + f)
    chunks = []
    mem_off = [0] * NT
    base = 0
    for ci in range(len(LD_BOUNDS) - 1):
        t0, t1 = LD_BOUNDS[ci], LD_BOUNDS[ci + 1]
        i0, i1 = inc_off[t0], inc_off[t1]
        mem_b = base + ((4 + 256) if ci == 0 else 0)  # chunk 0: scale + ident
        pos = mem_b
        for t in range(t0, t1):
            mem_off[t] = pos
            pos += 512 if _act_route(t) else 256
        qv_b = pos
        oh_b = qv_b + (i1 - i0) * 256
        end = oh_b + (i1 - i0) * 128
        chunks.append((mem_b, qv_b, oh_b, end))
        base = end
    return base, chunks, mem_off, inc_off


def build_nc(Fs):
    """Build the per-core Bass program.

    Fs: per slot-tile fragment counts (ceil(max-count-over-cores / 128)),
    shared across cores so one program serves all 8.
    """
    import concourse.bacc as bacc
    import concourse.tile as tile
    from concourse import mybir
    from contextlib import ExitStack

    f32 = mybir.dt.float32
    bf16 = mybir.dt.bfloat16
    fp8 = mybir.dt.float8e4
    u8 = mybir.dt.uint8
    i8 = mybir.dt.int8
    Alu = mybir.AluOpType

    TOT, chunks, mem_off, inc_off = _layout(Fs)
    assert all(b % GT == 0 for b in LD_BOUNDS + ST_BOUNDS)

    nc = bacc.Bacc("TRN2", target_bir_lowering=False, debug=False)

    img_in = nc.dram_tensor("img", [P, TOT], u8, kind="ExternalInput")
    out_kv = nc.dram_tensor("out_kv", [P, NT * 256], i8, kind="ExternalOutput")

    # view helpers: tile t lives in chunk ch(t); incidence inc in chunk of its tile
    def chunk_of(t):
        for ci in range(len(LD_BOUNDS) - 1):
            if LD_BOUNDS[ci] <= t < LD_BOUNDS[ci + 1]:
                return ci
        raise AssertionError

    with tile.TileContext(nc) as tc, ExitStack() as ctx:
        pool = ctx.enter_context(tc.tile_pool(name="main", bufs=1))
        pspool = ctx.enter_context(tc.tile_pool(name="ps", bufs=4, space="PSUM"))

        img_t = pool.tile([P, TOT], u8)
        out_t = pool.tile([P, NT * 256], i8)

        prev = 0
        for (mem_b, qv_b, oh_b, end) in chunks:
            nc.sync.dma_start(img_t[:, prev:end], img_in[:, prev:end])
            prev = end
        scale_ap = img_t[:, 0:4].bitcast(f32)      # [p, 1] = MEM_SCALE
        ident_t = img_t[:, 4:260].bitcast(bf16)    # [p, 128] identity

        def mem_view(t, n=1):
            # n tiles starting at t; all same route (route is per-group)
            off = mem_off[t]
            if _act_route(t):
                return img_t[:, off:off + n * 512].bitcast(bf16)
            return img_t[:, off:off + n * 256].bitcast(i8)

        def qv_view(t, fi):
            ci = chunk_of(t)
            qv_b = chunks[ci][1]
            off = qv_b + (inc_off[t] + fi - inc_off[LD_BOUNDS[ci]]) * 256
            return img_t[:, off:off + 256].bitcast(fp8)

        def oh_view(t, fi):
            ci = chunk_of(t)
            oh_b = chunks[ci][2]
            off = oh_b + (inc_off[t] + fi - inc_off[LD_BOUNDS[ci]]) * 128
            return img_t[:, off:off + 128].bitcast(fp8)

        st_done = 0
        for g in range(NT // GT):
            # one PSUM bank per tile ("start" zeroing operates on the whole
            # bank, so accumulation tiles must not share banks)
            act_route = _act_route(g * GT)
            ps = pspool.tile([P, GT * 512], f32, tag="ps")
            ps3 = ps[:].rearrange("p (i c) -> p i c", c=512)
            for i in range(GT):
                t = g * GT + i
                slc = ps[:, i * 512:i * 512 + 256]
                for fi in range(Fs[t]):
                    nc.tensor.matmul(
                        slc, lhsT=oh_view(t, fi), rhs=qv_view(t, fi),
                        start=(fi == 0),
                        stop=(not act_route and fi == Fs[t] - 1),
                    )
            c0 = g * GT * 256
            dst = out_t[:, c0:c0 + GT * 256].rearrange("p (i c) -> p i c", c=256)
            if act_route:
                # bf16 mem rides the PE, two tiles per strided matmul
                for h in range(GT // 2):
                    nc.tensor.matmul(
                        ps3[:, 2 * h:2 * h + 2, 0:256], lhsT=ident_t,
                        rhs=mem_view(g * GT + 2 * h, 2),
                        start=False, stop=True,
                    )
                nc.scalar.copy(dst, ps3[:, :, 0:256])
            else:
                # drain: out = s * mem_i8 + psum, fused on the DVE
                memv = mem_view(g * GT, GT).rearrange("p (i c) -> p i c", c=256)
                nc.vector.scalar_tensor_tensor(
                    dst, memv, 1.0, ps3[:, :, 0:256],
                    op0=Alu.mult, op1=Alu.add)

            tend = (g + 1) * GT
            if st_done < len(ST_BOUNDS) - 1 and tend == ST_BOUNDS[st_done + 1]:
                t0, t1 = ST_BOUNDS[st_done], ST_BOUNDS[st_done + 1]
                nc.scalar.dma_start(
                    out_kv[:, t0 * 256:t1 * 256], out_t[:, t0 * 256:t1 * 256])
                st_done += 1

    nc.compile()
    return nc


def prepare_inputs(inputs):
    """Host-side routing (the all-to-all stand-in): bucket contributions by
    (core, slot-tile), pre-divide weights by the local per-slot weight sums,
    and materialize each core's combined SBUF-image buffer."""
    import ml_dtypes
    bf16 = ml_dtypes.bfloat16
    fp8 = ml_dtypes.float8_e4m3

    mk = np.asarray(inputs["memory_keys"], dtype=np.float32)
    mv = np.asarray(inputs["memory_values"], dtype=np.float32)
    q = np.asarray(inputs["write_query"], dtype=np.float32)
    v = np.asarray(inputs["write_value"], dtype=np.float32)
    gate = np.asarray(inputs["gate_weights"], dtype=np.float32)
    ti = np.asarray(inputs["top_indices"]).astype(np.int64).reshape(-1)

    w = np.where(gate > GATE_THRESH, gate * np.float32(UPDATE_RATE),
                 np.float32(0.0)).astype(np.float32)
    wk = np.repeat(w, K)                                     # [B*K]
    cnt = np.bincount(ti, weights=wk.astype(np.float64),
                      minlength=N_SLOTS).astype(np.float32)
    denom = np.where(cnt > 0, cnt, np.float32(1.0)).astype(np.float32)
    # extra 1/MEM_SCALE so PSUM accumulates upd/s (int8 output units)
    ohv = (np.float32(UPD / MEM_SCALE) * wk / denom[ti]).astype(np.float32)

    a = np.arange(B * K, dtype=np.int64) // K
    gtile = ti >> 7                                          # global tile id
    order = np.argsort(gtile, kind="stable")
    g_s = gtile[order]
    a_s = a[order]
    s_s = (ti & 127)[order]
    ohv_s = ohv[order]
    cnt_pt = np.bincount(gtile, minlength=NCORES * NT)
    starts = np.zeros(NCORES * NT + 1, dtype=np.int64)
    starts[1:] = np.cumsum(cnt_pt)
    rowpos = np.arange(B * K, dtype=np.int64) - starts[g_s]

    cnt2 = cnt_pt.reshape(NCORES, NT)
    cnt_max = cnt2.max(axis=0)
    Fs = tuple(int(max(1, -(-c // 128))) for c in cnt_max)
    inc_off = np.zeros(NT + 1, dtype=np.int64)
    inc_off[1:] = np.cumsum(Fs)
    NINC = int(inc_off[-1])

    core_s = g_s >> 6
    t_s = g_s & 63
    inc_s = inc_off[t_s] + (rowpos >> 7)
    p_s = rowpos & 127

    qv_full = np.concatenate([q, v], axis=1)                 # [B, 256]
    qv_img = np.zeros((NCORES, P, NINC * 256), dtype=np.float32)
    oh_img = np.zeros((NCORES, P, NINC * 128), dtype=np.float32)
    oh_img[core_s, p_s, inc_s * 128 + s_s] = ohv_s
    cols = (inc_s * 256)[:, None] + np.arange(256)[None, :]
    qv_img[core_s[:, None], p_s[:, None], cols] = qv_full[a_s]
    qv_u8 = qv_img.astype(fp8).view(np.uint8)                # [C, P, NINC*256]
    oh_u8 = oh_img.astype(fp8).view(np.uint8)                # [C, P, NINC*128]

    mkv = np.concatenate([mk, mv], axis=1)                   # [65536, 256]
    # per-tile [C, P, 256] views in both encodings
    mem_t = np.ascontiguousarray(
        mkv.reshape(NCORES, NT, P, 256).transpose(0, 2, 1, 3))  # [C,P,NT,256]
    # int8 encoding (DVE route); |mem| > 6 clips, patched on host in kernel()
    mem_i8 = np.clip(np.round(mem_t / np.float32(MEM_SCALE)), -127, 127
                     ).astype(np.int8).view(np.uint8)
    mem_b16 = (mem_t / np.float32(MEM_SCALE)).astype(bf16).view(np.uint8)

    scale_u8 = np.broadcast_to(
        np.full((1, P, 1), MEM_SCALE, dtype=np.float32).view(np.uint8),
        (NCORES, P, 4))
    ident_u8 = np.broadcast_to(
        np.eye(P, dtype=bf16).view(np.uint8)[None], (NCORES, P, 256))
    parts = [scale_u8, ident_u8]
    for ci in range(len(LD_BOUNDS) - 1):
        t0, t1 = LD_BOUNDS[ci], LD_BOUNDS[ci + 1]
        i0, i1 = int(inc_off[t0]), int(inc_off[t1])
        for t in range(t0, t1):
            parts.append(mem_b16[:, :, t] if _act_route(t) else mem_i8[:, :, t])
        parts.append(qv_u8[:, :, i0 * 256:i1 * 256])
        parts.append(oh_u8[:, :, i0 * 128:i1 * 128])
    img = np.concatenate(parts, axis=2)                      # [C, P, TOT]

    in_maps = []
    for c in range(NCORES):
        in_maps.append({
            "img": np.ascontiguousarray(img[c]),
        })
    return in_maps, Fs


def kernel(**inputs):
    from concourse.bass_utils import run_bass_kernel_spmd

    in_maps, Fs = prepare_inputs(inputs)
    if Fs not in _BUILD_CACHE:
        _BUILD_CACHE[Fs] = build_nc(Fs)
    nc = _BUILD_CACHE[Fs]

    res = run_bass_kernel_spmd(nc, in_maps, core_ids=list(range(NCORES)))
    out_img = np.stack([res.results[c]["out_kv"] for c in range(NCORES)])
    # un-permute the SBUF image layout: [c, p, t*256+d] -> [c*8192+t*128+p, d]
    # and decode the int8 output units
    out_kv = np.ascontiguousarray(
        out_img.reshape(NCORES, P, NT, 256).transpose(0, 2, 1, 3)
    ).reshape(N_SLOTS, 256).astype(np.float32) * np.float32(MEM_SCALE)
    # sparse clip patch: int8 encoding clips |mem| > 6; restore those entries
    # (out = mem + upd is linear in mem, so adding the clip residual is exact)
    mk = np.asarray(inputs["memory_keys"], dtype=np.float32)
    mv = np.asarray(inputs["memory_values"], dtype=np.float32)
    mkv = np.concatenate([mk, mv], axis=1)
    rows = np.unique(np.nonzero(np.abs(mkv) > 126.5 * MEM_SCALE)[0])
    rows = rows[~np.vectorize(_act_route)(rows // 128 % NT)] if rows.size else rows
    if rows.size:
        dec = np.float32(MEM_SCALE) * np.clip(
            np.round(mkv[rows] / np.float32(MEM_SCALE)), -127, 127)
        out_kv[rows] += mkv[rows] - dec

    out_k = np.ascontiguousarray(out_kv[:, 0:DIM])
    out_v = np.ascontiguousarray(out_kv[:, DIM:2 * DIM])

    km = np.asarray(inputs["key_momentum"], dtype=np.float32)
    vm = np.asarray(inputs["value_momentum"], dtype=np.float32)
    # mom is zeros in this problem; fall back to a host-side add if it isn't
    if np.any(km):
        out_k = out_k + np.float32(MOMENTUM) * km
    if np.any(vm):
        out_v = out_v + np.float32(MOMENTUM) * vm
    return out_k, out_v


# revision 38
# speedup vs baseline: 1.2184x; 1.1685x over previous
"""Trainium2 Bass kernel for nn_MemoryWriter (scatter_memory).

Math (see reference):
    w        = where(gate > 0.01, gate * 0.1, 0)            [B]
    contrib  (q_a, v_a, w_a) scattered to slots top_indices[a, :]
    upd_k[s] = sum_j w_j q_j / (counts>0 ? counts : 1), counts = sum_j w_j
    out_k    = mem_k + 0.9 * mom_k + (1 - 0.9) * upd_k      (mom is zeros)

Sharding: slot dimension across 8 cores (8192 slots each).  The host performs
the contribution routing that the all-to-all performs in a real distributed
setting (per the sharding hint).  Because each slot lives on exactly one core,
the per-slot weight sums (counts) are host-computable during routing, so the
routed scatter weights are PRE-DIVIDED: oh[r, s] = (1-momentum) * w_r / denom_s.
The device work per 128-slot tile is then just:

    psum = oh_inc.T @ qv_inc  (+ further fragments)    # PE fp8 matmul scatter
    out_tile = psum + mem_tile  -> bf16                # drain+add

The drain is split per 4-tile PSUM group: tiles 0-1 drain on the DVE
(tensor_tensor add straight from PSUM), tiles 2-3 get the mem tile added by
an identity matmul on the PE and drain via an ACT copy — balancing PE/DVE/ACT.

All device inputs are packed host-side into ONE DRAM buffer per core laid out
as the exact SBUF image [128 partitions, bytes] = per chunk [mem|qv|oh], so
the whole input side is 5 large fully-contiguous DMAs.  The memory table
flows through the device in bf16 (rel err ~2e-3, tolerance 2e-2);
contributions in fp8e4m3; the host casts the bf16 output table back to f32.
"""

import numpy as np

# ---- problem constants (hardcoded per contest contract) --------------------
N_SLOTS = 65536
DIM = 128
B = 4096
K = 8
NCORES = 8
SPC = N_SLOTS // NCORES      # slots per core = 8192
NT = SPC // 128              # slot tiles per core = 64
P = 128
GATE_THRESH = 0.01
UPDATE_RATE = 0.1
MOMENTUM = 0.9
UPD = float(np.float32(1.0) - np.float32(MOMENTUM))

GT = 2                       # slot tiles per PSUM group (one bank per tile)
LD_BOUNDS = [0, 4, 12, 28, 44, 60, 64]   # load-chunk tile boundaries
ST_BOUNDS = [0, 16, 32, 48, 60, 64]      # store-chunk tile boundaries (%GT==0)
MEM_SCALE = 6.0 / 127.0      # int8 memory-table encoding: mem ~= s * q

_BUILD_CACHE = {}


def _act_route(t):
    """Drain routing: 18 of 32 groups on the DVE (int8 mem, fused add);
    14 on ACT (bf16 mem added by PE identity matmul, plain copy drain) —
    balances the two drain engines' ~0.82us-per-group cost."""
    return ((t // GT) % 16) in (1, 3, 5, 8, 10, 12, 14)


def _layout(Fs):
    """Byte layout of the combined per-core input image.

    Per load chunk: [scale/ident (chunk 0) | mem (256B int8 or 512B bf16 per
    tile, by drain route) | qv 256B/inc | oh 128B/inc] per partition.
    Returns (total_bytes, per-chunk bases, mem_off per tile, inc_off).
    """
    inc_off = [0]
    for f in Fs:
        inc_off.append(inc_off[-1] + f)
    chunks = []
    mem_off = [0] * NT
    base = 0
    for ci in range(len(LD_BOUNDS) - 1):
        t0, t1 = LD_BOUNDS[ci], LD_BOUNDS[ci + 1]
        i0, i1 = inc_off[t0], inc_off[t1]
        mem_b = base + ((4 + 256) if ci == 0 else 0)  # chunk 0: scale + ident
        pos = mem_b
        for t in range(t0, t1):
            mem_off[t] = pos
            pos += 512 if _act_route(t) else 256
        qv_b = pos
        oh_b = qv_b + (i1 - i0) * 256
        end = oh_b + (i1 - i0) * 128
        chunks.append((mem_b, qv_b, oh_b, end))
        base = end
    return base, chunks, mem_off, inc_off


def build_nc(Fs):
    """Build the per-core Bass program.

    Fs: per slot-tile fragment counts (ceil(max-count-over-cores / 128)),
    shared across cores so one program serves all 8.
    """
    import concourse.bacc as bacc
    import concourse.tile as tile
    from concourse import mybir
    from contextlib import ExitStack

    f32 = mybir.dt.float32
    bf16 = mybir.dt.bfloat16
    fp8 = mybir.dt.float8e4
    u8 = mybir.dt.uint8
    i8 = mybir.dt.int8
    Alu = mybir.AluOpType

    TOT, chunks, mem_off, inc_off = _layout(Fs)
    assert all(b % GT == 0 for b in LD_BOUNDS + ST_BOUNDS)

    nc = bacc.Bacc("TRN2", target_bir_lowering=False, debug=False)

    img_in = nc.dram_tensor("img", [P, TOT], u8, kind="ExternalInput")
    out_kv = nc.dram_tensor("out_kv", [P, NT * 256], i8, kind="ExternalOutput")

    # view helpers: tile t lives in chunk ch(t); incidence inc in chunk of its tile
    def chunk_of(t):
        for ci in range(len(LD_BOUNDS) - 1):
            if LD_BOUNDS[ci] <= t < LD_BOUNDS[ci + 1]:
                return ci
        raise AssertionError

    with tile.TileContext(nc) as tc, ExitStack() as ctx:
        pool = ctx.enter_context(tc.tile_pool(name="main", bufs=1))
        pspool = ctx.enter_context(tc.tile_pool(name="ps", bufs=4, space="PSUM"))

        img_t = pool.tile([P, TOT], u8)
        out_t = pool.tile([P, NT * 256], i8)

        prev = 0
        for (mem_b, qv_b, oh_b, end) in chunks:
            nc.sync.dma_start(img_t[:, prev:end], img_in[:, prev:end])
            prev = end
        scale_ap = img_t[:, 0:4].bitcast(f32)      # [p, 1] = MEM_SCALE
        ident_t = img_t[:, 4:260].bitcast(bf16)    # [p, 128] identity

        def mem_view(t, n=1):
            # n tiles starting at t; all same route (route is per-group)
            off = mem_off[t]
            if _act_route(t):
                return img_t[:, off:off + n * 512].bitcast(bf16)
            return img_t[:, off:off + n * 256].bitcast(i8)

        def qv_view(t, fi):
            ci = chunk_of(t)
            qv_b = chunks[ci][1]
            off = qv_b + (inc_off[t] + fi - inc_off[LD_BOUNDS[ci]]) * 256
            return img_t[:, off:off + 256].bitcast(fp8)

        def oh_view(t, fi):
            ci = chunk_of(t)
            oh_b = chunks[ci][2]
            off = oh_b + (inc_off[t] + fi - inc_off[LD_BOUNDS[ci]]) * 128
            return img_t[:, off:off + 128].bitcast(fp8)

        st_done = 0
        for g in range(NT // GT):
            # one PSUM bank per tile ("start" zeroing operates on the whole
            # bank, so accumulation tiles must not share banks)
            act_route = _act_route(g * GT)
            ps = pspool.tile([P, GT * 512], f32, tag="ps")
            ps3 = ps[:].rearrange("p (i c) -> p i c", c=512)
            for i in range(GT):
                t = g * GT + i
                slc = ps[:, i * 512:i * 512 + 256]
                for fi in range(Fs[t]):
                    nc.tensor.matmul(
                        slc, lhsT=oh_view(t, fi), rhs=qv_view(t, fi),
                        start=(fi == 0),
                        stop=(not act_route and fi == Fs[t] - 1),
                    )
            c0 = g * GT * 256
            dst = out_t[:, c0:c0 + GT * 256].rearrange("p (i c) -> p i c", c=256)
            if act_route:
                # bf16 mem rides the PE, two tiles per strided matmul
                for h in range(GT // 2):
                    nc.tensor.matmul(
                        ps3[:, 2 * h:2 * h + 2, 0:256], lhsT=ident_t,
                        rhs=mem_view(g * GT + 2 * h, 2),
                        start=False, stop=True,
                    )
                nc.scalar.copy(dst, ps3[:, :, 0:256])
            else:
                # drain: out = s * mem_i8 + psum, fused on the DVE
                memv = mem_view(g * GT, GT).rearrange("p (i c) -> p i c", c=256)
                nc.vector.scalar_tensor_tensor(
                    dst, memv, 1.0, ps3[:, :, 0:256],
                    op0=Alu.mult, op1=Alu.add)

            tend = (g + 1) * GT
            if st_done < len(ST_BOUNDS) - 1 and tend == ST_BOUNDS[st_done + 1]:
                t0, t1 = ST_BOUNDS[st_done], ST_BOUNDS[st_done + 1]
                nc.sync.dma_start(
                    out_kv[:, t0 * 256:t1 * 256], out_t[:, t0 * 256:t1 * 256])
                st_done += 1

    nc.compile()
    return nc


def prepare_inputs(inputs):
    """Host-side routing (the all-to-all stand-in): bucket contributions by
    (core, slot-tile), pre-divide weights by the local per-slot weight sums,
    and materialize each core's combined SBUF-image buffer."""
    import ml_dtypes
    bf16 = ml_dtypes.bfloat16
    fp8 = ml_dtypes.float8_e4m3

    mk = np.asarray(inputs["memory_keys"], dtype=np.float32)
    mv = np.asarray(inputs["memory_values"], dtype=np.float32)
    q = np.asarray(inputs["write_query"], dtype=np.float32)
    v = np.asarray(inputs["write_value"], dtype=np.float32)
    gate = np.asarray(inputs["gate_weights"], dtype=np.float32)
    ti = np.asarray(inputs["top_indices"]).astype(np.int64).reshape(-1)

    w = np.where(gate > GATE_THRESH, gate * np.float32(UPDATE_RATE),
                 np.float32(0.0)).astype(np.float32)
    wk = np.repeat(w, K)                                     # [B*K]
    cnt = np.bincount(ti, weights=wk.astype(np.float64),
                      minlength=N_SLOTS).astype(np.float32)
    denom = np.where(cnt > 0, cnt, np.float32(1.0)).astype(np.float32)
    # extra 1/MEM_SCALE so PSUM accumulates upd/s (int8 output units)
    ohv = (np.float32(UPD / MEM_SCALE) * wk / denom[ti]).astype(np.float32)

    a = np.arange(B * K, dtype=np.int64) // K
    gtile = ti >> 7                                          # global tile id
    order = np.argsort(gtile, kind="stable")
    g_s = gtile[order]
    a_s = a[order]
    s_s = (ti & 127)[order]
    ohv_s = ohv[order]
    cnt_pt = np.bincount(gtile, minlength=NCORES * NT)
    starts = np.zeros(NCORES * NT + 1, dtype=np.int64)
    starts[1:] = np.cumsum(cnt_pt)
    rowpos = np.arange(B * K, dtype=np.int64) - starts[g_s]

    cnt2 = cnt_pt.reshape(NCORES, NT)
    cnt_max = cnt2.max(axis=0)
    Fs = tuple(int(max(1, -(-c // 128))) for c in cnt_max)
    inc_off = np.zeros(NT + 1, dtype=np.int64)
    inc_off[1:] = np.cumsum(Fs)
    NINC = int(inc_off[-1])

    core_s = g_s >> 6
    t_s = g_s & 63
    inc_s = inc_off[t_s] + (rowpos >> 7)
    p_s = rowpos & 127

    qv_full = np.concatenate([q, v], axis=1)                 # [B, 256]
    qv_img = np.zeros((NCORES, P, NINC * 256), dtype=np.float32)
    oh_img = np.zeros((NCORES, P, NINC * 128), dtype=np.float32)
    oh_img[core_s, p_s, inc_s * 128 + s_s] = ohv_s
    cols = (inc_s * 256)[:, None] + np.arange(256)[None, :]
    qv_img[core_s[:, None], p_s[:, None], cols] = qv_full[a_s]
    qv_u8 = qv_img.astype(fp8).view(np.uint8)                # [C, P, NINC*256]
    oh_u8 = oh_img.astype(fp8).view(np.uint8)                # [C, P, NINC*128]

    mkv = np.concatenate([mk, mv], axis=1)                   # [65536, 256]
    # per-tile [C, P, 256] views in both encodings
    mem_t = np.ascontiguousarray(
        mkv.reshape(NCORES, NT, P, 256).transpose(0, 2, 1, 3))  # [C,P,NT,256]
    # int8 encoding (DVE route); |mem| > 6 clips, patched on host in kernel()
    mem_i8 = np.clip(np.round(mem_t / np.float32(MEM_SCALE)), -127, 127
                     ).astype(np.int8).view(np.uint8)
    mem_b16 = (mem_t / np.float32(MEM_SCALE)).astype(bf16).view(np.uint8)

    scale_u8 = np.broadcast_to(
        np.full((1, P, 1), MEM_SCALE, dtype=np.float32).view(np.uint8),
        (NCORES, P, 4))
    ident_u8 = np.broadcast_to(
        np.eye(P, dtype=bf16).view(np.uint8)[None], (NCORES, P, 256))
    parts = [scale_u8, ident_u8]
    for ci in range(len(LD_BOUNDS) - 1):
        t0, t1 = LD_BOUNDS[ci], LD_BOUNDS[ci + 1]
        i0, i1 = int(inc_off[t0]), int(inc_off[t1])
        for t in range(t0, t1):
            parts.append(mem_b16[:, :, t] if _act_route(t) else mem_i8[:, :, t])
        parts.append(qv_u8[:, :, i0 * 256:i1 * 256])
        parts.append(oh_u8[:, :, i0 * 128:i1 * 128])
    img = np.concatenate(parts, axis=2)                      # [C, P, TOT]

    in_maps = []
    for c in range(NCORES):
        in_maps.append({
            "img": np.ascontiguousarray(img[c]),
        })
    return in_maps, Fs


def kernel(**inputs):
    from concourse.bass_utils import run_bass_kernel_spmd

    in_maps, Fs = prepare_inputs(inputs)
    if Fs not in _BUILD_CACHE:
        _BUILD_CACHE[Fs] = build_nc(Fs)
    nc = _BUILD_CACHE[Fs]

    res = run_bass_kernel_spmd(nc, in_maps, core_ids=list(range(NCORES)))
    out_img = np.stack([res.results[c]["out_kv"] for c in range(NCORES)])
    # un-permute the SBUF image layout: [c, p, t*256+d] -> [c*8192+t*128+p, d]
    # and decode the int8 output units
    out_kv = np.ascontiguousarray(
        out_img.reshape(NCORES, P, NT, 256).transpose(0, 2, 1, 3)
    ).reshape(N_SLOTS, 256).astype(np.float32) * np.float32(MEM_SCALE)
    # sparse clip patch: int8 encoding clips |mem| > 6; restore those entries
    # (out = mem + upd is linear in mem, so adding the clip residual is exact)
    mk = np.asarray(inputs["memory_keys"], dtype=np.float32)
    mv = np.asarray(inputs["memory_values"], dtype=np.float32)
    mkv = np.concatenate([mk, mv], axis=1)
    rows = np.unique(np.nonzero(np.abs(mkv) > 126.5 * MEM_SCALE)[0])
    rows = rows[~np.vectorize(_act_route)(rows // 128 % NT)] if rows.size else rows
    if rows.size:
        dec = np.float32(MEM_SCALE) * np.clip(
            np.round(mkv[rows] / np.float32(MEM_SCALE)), -127, 127)
        out_kv[rows] += mkv[rows] - dec

    out_k = np.ascontiguousarray(out_kv[:, 0:DIM])
    out_v = np.ascontiguousarray(out_kv[:, DIM:2 * DIM])

    km = np.asarray(inputs["key_momentum"], dtype=np.float32)
    vm = np.asarray(inputs["value_momentum"], dtype=np.float32)
    # mom is zeros in this problem; fall back to a host-side add if it isn't
    if np.any(km):
        out_k = out_k + np.float32(MOMENTUM) * km
    if np.any(vm):
        out_v = out_v + np.float32(MOMENTUM) * vm
    return out_k, out_v


# revision 39
# speedup vs baseline: 1.2481x; 1.0243x over previous
"""Trainium2 Bass kernel for nn_MemoryWriter (scatter_memory).

Math (see reference):
    w        = where(gate > 0.01, gate * 0.1, 0)            [B]
    contrib  (q_a, v_a, w_a) scattered to slots top_indices[a, :]
    upd_k[s] = sum_j w_j q_j / (counts>0 ? counts : 1), counts = sum_j w_j
    out_k    = mem_k + 0.9 * mom_k + (1 - 0.9) * upd_k      (mom is zeros)

Sharding: slot dimension across 8 cores (8192 slots each).  The host performs
the contribution routing that the all-to-all performs in a real distributed
setting (per the sharding hint).  Because each slot lives on exactly one core,
the per-slot weight sums (counts) are host-computable during routing, so the
routed scatter weights are PRE-DIVIDED: oh[r, s] = (1-momentum) * w_r / denom_s.
The device work per 128-slot tile is then just:

    psum = oh_inc.T @ qv_inc  (+ further fragments)    # PE fp8 matmul scatter
    out_tile = psum + mem_tile  -> bf16                # drain+add

The drain is split per 4-tile PSUM group: tiles 0-1 drain on the DVE
(tensor_tensor add straight from PSUM), tiles 2-3 get the mem tile added by
an identity matmul on the PE and drain via an ACT copy — balancing PE/DVE/ACT.

All device inputs are packed host-side into ONE DRAM buffer per core laid out
as the exact SBUF image [128 partitions, bytes] = per chunk [mem|qv|oh], so
the whole input side is 5 large fully-contiguous DMAs.  The memory table
flows through the device in bf16 (rel err ~2e-3, tolerance 2e-2);
contributions in fp8e4m3; the host casts the bf16 output table back to f32.
"""

import numpy as np

# ---- problem constants (hardcoded per contest contract) --------------------
N_SLOTS = 65536
DIM = 128
B = 4096
K = 8
NCORES = 8
SPC = N_SLOTS // NCORES      # slots per core = 8192
NT = SPC // 128              # slot tiles per core = 64
P = 128
GATE_THRESH = 0.01
UPDATE_RATE = 0.1
MOMENTUM = 0.9
UPD = float(np.float32(1.0) - np.float32(MOMENTUM))

GT = 2                       # slot tiles per PSUM group (one bank per tile)
LD_BOUNDS = [0, 2, 6, 14, 28, 42, 56, 64]  # load-chunk tile boundaries
ST_BOUNDS = [0, 16, 32, 48, 60, 64]      # store-chunk tile boundaries (%GT==0)
MEM_SCALE = 6.0 / 127.0      # int8 memory-table encoding: mem ~= s * q

_BUILD_CACHE = {}


def _act_route(t):
    """Drain routing: 18 of 32 groups on the DVE (int8 mem, fused add);
    14 on ACT (bf16 mem added by PE identity matmul, plain copy drain) —
    balances the two drain engines' ~0.82us-per-group cost."""
    return ((t // GT) % 16) in (1, 3, 5, 8, 10, 12, 14)


def _layout(Fs):
    """Byte layout of the combined per-core input image.

    Per load chunk: [scale/ident (chunk 0) | mem (256B int8 or 512B bf16 per
    tile, by drain route) | qv 256B/inc | oh 128B/inc] per partition.
    Returns (total_bytes, per-chunk bases, mem_off per tile, inc_off).
    """
    inc_off = [0]
    for f in Fs:
        inc_off.append(inc_off[-1] + f)
    chunks = []
    mem_off = [0] * NT
    base = 0
    for ci in range(len(LD_BOUNDS) - 1):
        t0, t1 = LD_BOUNDS[ci], LD_BOUNDS[ci + 1]
        i0, i1 = inc_off[t0], inc_off[t1]
        mem_b = base + ((4 + 256) if ci == 0 else 0)  # chunk 0: scale + ident
        pos = mem_b
        for t in range(t0, t1):
            mem_off[t] = pos
            pos += 512 if _act_route(t) else 256
        qv_b = pos
        oh_b = qv_b + (i1 - i0) * 256
        end = oh_b + (i1 - i0) * 128
        chunks.append((mem_b, qv_b, oh_b, end))
        base = end
    return base, chunks, mem_off, inc_off


def build_nc(Fs):
    """Build the per-core Bass program.

    Fs: per slot-tile fragment counts (ceil(max-count-over-cores / 128)),
    shared across cores so one program serves all 8.
    """
    import concourse.bacc as bacc
    import concourse.tile as tile
    from concourse import mybir
    from contextlib import ExitStack

    f32 = mybir.dt.float32
    bf16 = mybir.dt.bfloat16
    fp8 = mybir.dt.float8e4
    u8 = mybir.dt.uint8
    i8 = mybir.dt.int8
    Alu = mybir.AluOpType

    TOT, chunks, mem_off, inc_off = _layout(Fs)
    assert all(b % GT == 0 for b in LD_BOUNDS + ST_BOUNDS)

    nc = bacc.Bacc("TRN2", target_bir_lowering=False, debug=False)

    img_in = nc.dram_tensor("img", [P, TOT], u8, kind="ExternalInput")
    out_kv = nc.dram_tensor("out_kv", [P, NT * 256], i8, kind="ExternalOutput")

    # view helpers: tile t lives in chunk ch(t); incidence inc in chunk of its tile
    def chunk_of(t):
        for ci in range(len(LD_BOUNDS) - 1):
            if LD_BOUNDS[ci] <= t < LD_BOUNDS[ci + 1]:
                return ci
        raise AssertionError

    with tile.TileContext(nc) as tc, ExitStack() as ctx:
        pool = ctx.enter_context(tc.tile_pool(name="main", bufs=1))
        pspool = ctx.enter_context(tc.tile_pool(name="ps", bufs=4, space="PSUM"))

        img_t = pool.tile([P, TOT], u8)
        out_t = pool.tile([P, NT * 256], i8)

        prev = 0
        for (mem_b, qv_b, oh_b, end) in chunks:
            nc.sync.dma_start(img_t[:, prev:end], img_in[:, prev:end])
            prev = end
        scale_ap = img_t[:, 0:4].bitcast(f32)      # [p, 1] = MEM_SCALE
        ident_t = img_t[:, 4:260].bitcast(bf16)    # [p, 128] identity

        def mem_view(t, n=1):
            # n tiles starting at t; all same route (route is per-group)
            off = mem_off[t]
            if _act_route(t):
                return img_t[:, off:off + n * 512].bitcast(bf16)
            return img_t[:, off:off + n * 256].bitcast(i8)

        def qv_view(t, fi):
            ci = chunk_of(t)
            qv_b = chunks[ci][1]
            off = qv_b + (inc_off[t] + fi - inc_off[LD_BOUNDS[ci]]) * 256
            return img_t[:, off:off + 256].bitcast(fp8)

        def oh_view(t, fi):
            ci = chunk_of(t)
            oh_b = chunks[ci][2]
            off = oh_b + (inc_off[t] + fi - inc_off[LD_BOUNDS[ci]]) * 128
            return img_t[:, off:off + 128].bitcast(fp8)

        st_done = 0
        for g in range(NT // GT):
            # one PSUM bank per tile ("start" zeroing operates on the whole
            # bank, so accumulation tiles must not share banks)
            act_route = _act_route(g * GT)
            ps = pspool.tile([P, GT * 512], f32, tag="ps")
            ps3 = ps[:].rearrange("p (i c) -> p i c", c=512)
            for i in range(GT):
                t = g * GT + i
                slc = ps[:, i * 512:i * 512 + 256]
                for fi in range(Fs[t]):
                    nc.tensor.matmul(
                        slc, lhsT=oh_view(t, fi), rhs=qv_view(t, fi),
                        start=(fi == 0),
                        stop=(not act_route and fi == Fs[t] - 1),
                    )
            c0 = g * GT * 256
            dst = out_t[:, c0:c0 + GT * 256].rearrange("p (i c) -> p i c", c=256)
            if act_route:
                # bf16 mem rides the PE, two tiles per strided matmul
                for h in range(GT // 2):
                    nc.tensor.matmul(
                        ps3[:, 2 * h:2 * h + 2, 0:256], lhsT=ident_t,
                        rhs=mem_view(g * GT + 2 * h, 2),
                        start=False, stop=True,
                    )
                nc.scalar.copy(dst, ps3[:, :, 0:256])
            else:
                # drain: out = s * mem_i8 + psum, fused on the DVE
                memv = mem_view(g * GT, GT).rearrange("p (i c) -> p i c", c=256)
                nc.vector.scalar_tensor_tensor(
                    dst, memv, 1.0, ps3[:, :, 0:256],
                    op0=Alu.mult, op1=Alu.add)

            tend = (g + 1) * GT
            if st_done < len(ST_BOUNDS) - 1 and tend == ST_BOUNDS[st_done + 1]:
                t0, t1 = ST_BOUNDS[st_done], ST_BOUNDS[st_done + 1]
                nc.sync.dma_start(
                    out_kv[:, t0 * 256:t1 * 256], out_t[:, t0 * 256:t1 * 256])
                st_done += 1

    nc.compile()
    return nc


def prepare_inputs(inputs):
    """Host-side routing (the all-to-all stand-in): bucket contributions by
    (core, slot-tile), pre-divide weights by the local per-slot weight sums,
    and materialize each core's combined SBUF-image buffer."""
    import ml_dtypes
    bf16 = ml_dtypes.bfloat16
    fp8 = ml_dtypes.float8_e4m3

    mk = np.asarray(inputs["memory_keys"], dtype=np.float32)
    mv = np.asarray(inputs["memory_values"], dtype=np.float32)
    q = np.asarray(inputs["write_query"], dtype=np.float32)
    v = np.asarray(inputs["write_value"], dtype=np.float32)
    gate = np.asarray(inputs["gate_weights"], dtype=np.float32)
    ti = np.asarray(inputs["top_indices"]).astype(np.int64).reshape(-1)

    w = np.where(gate > GATE_THRESH, gate * np.float32(UPDATE_RATE),
                 np.float32(0.0)).astype(np.float32)
    wk = np.repeat(w, K)                                     # [B*K]
    cnt = np.bincount(ti, weights=wk.astype(np.float64),
                      minlength=N_SLOTS).astype(np.float32)
    denom = np.where(cnt > 0, cnt, np.float32(1.0)).astype(np.float32)
    # extra 1/MEM_SCALE so PSUM accumulates upd/s (int8 output units)
    ohv = (np.float32(UPD / MEM_SCALE) * wk / denom[ti]).astype(np.float32)

    a = np.arange(B * K, dtype=np.int64) // K
    gtile = ti >> 7                                          # global tile id
    order = np.argsort(gtile, kind="stable")
    g_s = gtile[order]
    a_s = a[order]
    s_s = (ti & 127)[order]
    ohv_s = ohv[order]
    cnt_pt = np.bincount(gtile, minlength=NCORES * NT)
    starts = np.zeros(NCORES * NT + 1, dtype=np.int64)
    starts[1:] = np.cumsum(cnt_pt)
    rowpos = np.arange(B * K, dtype=np.int64) - starts[g_s]

    cnt2 = cnt_pt.reshape(NCORES, NT)
    cnt_max = cnt2.max(axis=0)
    Fs = tuple(int(max(1, -(-c // 128))) for c in cnt_max)
    inc_off = np.zeros(NT + 1, dtype=np.int64)
    inc_off[1:] = np.cumsum(Fs)
    NINC = int(inc_off[-1])

    core_s = g_s >> 6
    t_s = g_s & 63
    inc_s = inc_off[t_s] + (rowpos >> 7)
    p_s = rowpos & 127

    qv_full = np.concatenate([q, v], axis=1)                 # [B, 256]
    qv_img = np.zeros((NCORES, P, NINC * 256), dtype=np.float32)
    oh_img = np.zeros((NCORES, P, NINC * 128), dtype=np.float32)
    oh_img[core_s, p_s, inc_s * 128 + s_s] = ohv_s
    cols = (inc_s * 256)[:, None] + np.arange(256)[None, :]
    qv_img[core_s[:, None], p_s[:, None], cols] = qv_full[a_s]
    qv_u8 = qv_img.astype(fp8).view(np.uint8)                # [C, P, NINC*256]
    oh_u8 = oh_img.astype(fp8).view(np.uint8)                # [C, P, NINC*128]

    mkv = np.concatenate([mk, mv], axis=1)                   # [65536, 256]
    # per-tile [C, P, 256] views in both encodings
    mem_t = np.ascontiguousarray(
        mkv.reshape(NCORES, NT, P, 256).transpose(0, 2, 1, 3))  # [C,P,NT,256]
    # int8 encoding (DVE route); |mem| > 6 clips, patched on host in kernel()
    mem_i8 = np.clip(np.round(mem_t / np.float32(MEM_SCALE)), -127, 127
                     ).astype(np.int8).view(np.uint8)
    mem_b16 = (mem_t / np.float32(MEM_SCALE)).astype(bf16).view(np.uint8)

    scale_u8 = np.broadcast_to(
        np.full((1, P, 1), MEM_SCALE, dtype=np.float32).view(np.uint8),
        (NCORES, P, 4))
    ident_u8 = np.broadcast_to(
        np.eye(P, dtype=bf16).view(np.uint8)[None], (NCORES, P, 256))
    parts = [scale_u8, ident_u8]
    for ci in range(len(LD_BOUNDS) - 1):
        t0, t1 = LD_BOUNDS[ci], LD_BOUNDS[ci + 1]
        i0, i1 = int(inc_off[t0]), int(inc_off[t1])
        for t in range(t0, t1):
            parts.append(mem_b16[:, :, t] if _act_route(t) else mem_i8[:, :, t])
        parts.append(qv_u8[:, :, i0 * 256:i1 * 256])
        parts.append(oh_u8[:, :, i0 * 128:i1 * 128])
    img = np.concatenate(parts, axis=2)                      # [C, P, TOT]

    in_maps = []
    for c in range(NCORES):
        in_maps.append({
            "img": np.ascontiguousarray(img[c]),
        })
    return in_maps, Fs


def kernel(**inputs):
    from concourse.bass_utils import run_bass_kernel_spmd

    in_maps, Fs = prepare_inputs(inputs)
    if Fs not in _BUILD_CACHE:
        _BUILD_CACHE[Fs] = build_nc(Fs)
    nc = _BUILD_CACHE[Fs]

    res = run_bass_kernel_spmd(nc, in_maps, core_ids=list(range(NCORES)))
    out_img = np.stack([res.results[c]["out_kv"] for c in range(NCORES)])
    # un-permute the SBUF image layout: [c, p, t*256+d] -> [c*8192+t*128+p, d]
    # and decode the int8 output units
    out_kv = np.ascontiguousarray(
        out_img.reshape(NCORES, P, NT, 256).transpose(0, 2, 1, 3)
    ).reshape(N_SLOTS, 256).astype(np.float32) * np.float32(MEM_SCALE)
    # sparse clip patch: int8 encoding clips |mem| > 6; restore those entries
    # (out = mem + upd is linear in mem, so adding the clip residual is exact)
    mk = np.asarray(inputs["memory_keys"], dtype=np.float32)
    mv = np.asarray(inputs["memory_values"], dtype=np.float32)
    mkv = np.concatenate([mk, mv], axis=1)
    rows = np.unique(np.nonzero(np.abs(mkv) > 126.5 * MEM_SCALE)[0])
    rows = rows[~np.vectorize(_act_route)(rows // 128 % NT)] if rows.size else rows
    if rows.size:
        dec = np.float32(MEM_SCALE) * np.clip(
            np.round(mkv[rows] / np.float32(MEM_SCALE)), -127, 127)
        out_kv[rows] += mkv[rows] - dec

    out_k = np.ascontiguousarray(out_kv[:, 0:DIM])
    out_v = np.ascontiguousarray(out_kv[:, DIM:2 * DIM])

    km = np.asarray(inputs["key_momentum"], dtype=np.float32)
    vm = np.asarray(inputs["value_momentum"], dtype=np.float32)
    # mom is zeros in this problem; fall back to a host-side add if it isn't
    if np.any(km):
        out_k = out_k + np.float32(MOMENTUM) * km
    if np.any(vm):
        out_v = out_v + np.float32(MOMENTUM) * vm
    return out_k, out_v
